# revision 12
# baseline (speedup 1.0000x reference)
"""Trainium2 Bass kernel for per-pixel cosine-distance block.

x1: [B, C, h, w]  f32
x2: [B, S, C, h, w] f32
out: [B, S*h*w] f32  where out[b, s*h*w + p] = 1 - cos(x1[b,:,p], x2[b,s,:,p])
(cosine over the channel dim C, per pixel)

Sharding: data-parallel over B across 8 NeuronCores (4 batches per core).

Per-core pipeline (C=512 on partitions as 4 chunks of 128, hw=1024 on free;
c = p*4+k so each partition's chunks are 16 KiB-contiguous in DRAM):

  All input DMAs are SWDGE (gpsimd) f32->bf16 cast loads on one queue at
  HBM line rate (~425 GB/s/core read side); the gpsimd engine issues
  nothing else, so the load queue is never blocked behind compute.
  Output stores ride the otherwise-idle sync HWDGE queue.

  x1 is normalized up front per batch (qn = x1 * rsqrt(sum x1^2), with the
  rsqrt row broadcast across partitions by a K=1 matmul), so the per-tile
  work is: prod = qn*x2 (DVE bf16 2x mode), sq2 = x2^2 (ScalarE; DVE for
  one s per batch to balance the two engines), and one-hot matmul
  partition-reductions into PSUM.

  TensorE: s-tile s uses PE column group g = s%4 (tile_position=(0,32g)).
  Group g accumulates rows [dot[g], dot[g+4], ss2[g], ss2[g+4]] at PSUM
  partitions 32g..32g+3 via an M=4 one-hot lhsT. Column groups execute
  concurrently when several tiles' matmuls are pending, so the PE keeps
  up with the DMA cadence even at its cold (1.2 GHz) clock, and each
  group's accumulation closes at s=g+4 - epilogues and output stores
  spread through the batch instead of piling up at the end. The final
  drain after the last byte lands is just group 3's short chain
  (rsqrt -> mul -> 1-x -> store), with the last tile loaded in two hw
  halves to shorten it further.
"""

from contextlib import ExitStack

import numpy as np

import concourse.bass as bass
import concourse.tile as tile
from concourse import bacc, mybir
from concourse.bass_utils import run_bass_kernel_spmd

B, S, C, H, W = 32, 8, 512, 32, 32
HW = H * W  # 1024
N_CORES = 8
BL = B // N_CORES  # 4 batches per core
P = 128
NCH = C // P  # 4 chunks of the channel dim
HWH = HW // 2  # 512 (one PSUM bank of f32)
NG = 4  # PE column groups; s-tile s -> group s % NG
SPG = S // NG  # s values per group (2)

FP32 = mybir.dt.float32
BF16 = mybir.dt.bfloat16

RSQRT = mybir.ActivationFunctionType.Abs_reciprocal_sqrt
SQUARE = mybir.ActivationFunctionType.Square


def _emit(ctx: ExitStack, tc: tile.TileContext, x1, x2, out):
    nc = tc.nc

    # c = p*NCH + k -> partition p, chunk k: 16 KiB contiguous per partition
    x1r = x1.rearrange("b (p k) f -> b p k f", p=P)  # [BL, 128, NCH, HW]
    x2r = x2.rearrange("b s (p k) f -> b s p k f", p=P)  # [BL, S, 128, NCH, HW]
    # s = j*NG + g -> out rows of group g hold s in {g, g+4}
    outr = out.rearrange("b (j g) f -> b g j f", g=NG)  # [BL, NG, SPG, HW]

    singles = ctx.enter_context(tc.tile_pool(name="singles", bufs=1))
    x1_pool = ctx.enter_context(tc.tile_pool(name="x1", bufs=2))
    qn_pool = ctx.enter_context(tc.tile_pool(name="qn", bufs=2))
    sq1_pool = ctx.enter_context(tc.tile_pool(name="sq1", bufs=1))
    x2_pool = ctx.enter_context(tc.tile_pool(name="x2", bufs=6))
    prod_pool = ctx.enter_context(tc.tile_pool(name="prod", bufs=4))
    sq2_pool = ctx.enter_context(tc.tile_pool(name="sq2", bufs=4))
    rr1_pool = ctx.enter_context(tc.tile_pool(name="rr1", bufs=1))
    rr2_pool = ctx.enter_context(tc.tile_pool(name="rr2", bufs=3))
    t_pool = ctx.enter_context(tc.tile_pool(name="tp", bufs=3))
    dist_pool = ctx.enter_context(tc.tile_pool(name="dist", bufs=2))
    # PSUM: 8 banks exactly = dot acc (2) + ss2 acc (2) + ss1 (2) + rep (2).
    # bufs=1 is safe: each group's accumulator is read (epilogue) at s=g+4
    # of batch b, ~4 tiles before batch b+1 rewrites it at its s=g.
    psum_pool = ctx.enter_context(tc.tile_pool(name="pacc", bufs=1, space="PSUM"))
    psum2_pool = ctx.enter_context(tc.tile_pool(name="pac2", bufs=1, space="PSUM"))
    ss1_pool = ctx.enter_context(tc.tile_pool(name="ss1p", bufs=1, space="PSUM"))
    rep_pool = ctx.enter_context(tc.tile_pool(name="rep", bufs=1, space="PSUM"))

    # oh2[:, j, :] is a [P, 2] matrix, all-ones in column j: as lhsT it
    # deposits the partition-reduction of rhs into row j of the 2-row
    # group region (adding zero to the other row).
    oh2 = singles.tile([P, SPG, SPG], BF16)
    nc.vector.memset(oh2, 0.0)
    for r in range(SPG):
        nc.vector.memset(oh2[:, r, r : r + 1], 1.0)
    ones1 = singles.tile([P, 1], BF16)
    nc.vector.memset(ones1, 1.0)
    # [1, P] ones: K=1 matmul with it as lhsT replicates an SBUF row across
    # all 128 PSUM partitions (fp32 so the values pass through unchanged).
    ones128 = singles.tile([1, P], FP32)
    nc.vector.memset(ones128, 1.0)

    def load_x1(b):
        x1_t = x1_pool.tile([P, NCH, HW], BF16)
        nc.gpsimd.dma_start(x1_t[:], x1r[b])
        return x1_t

    def qn_chain(b, x1_t):
        # qn = x1 * rsqrt(sum_c x1^2); also allocates batch b's group
        # accumulator (partitions 32g..32g+3: dot[g], dot[g+4], ss2[g],
        # ss2[g+4], each [2 banks] = both hw halves).
        sq1 = sq1_pool.tile([P, NCH, HW], BF16)
        nc.vector.tensor_mul(sq1[:], x1_t[:], x1_t[:])
        ss1 = ss1_pool.tile([1, 2, HWH], FP32)
        for hh in range(2):
            for ic in range(NCH):
                nc.tensor.matmul(
                    ss1[:, hh, :],
                    ones1,
                    sq1[:, ic, hh * HWH : (hh + 1) * HWH],
                    start=(ic == 0),
                    stop=(ic == NCH - 1),
                )
        rr1 = rr1_pool.tile([1, 2, HWH], FP32)
        nc.scalar.activation(rr1[:], ss1[:], func=RSQRT)
        rep = rep_pool.tile([P, 2, HWH], FP32)  # 2 banks
        for hh in range(2):
            nc.tensor.matmul(
                rep[:, hh, :], ones128, rr1[:, hh, :], start=True, stop=True
            )
        qn = qn_pool.tile([P, NCH, HW], BF16)
        for ic in range(NCH):
            for hh in range(2):
                nc.vector.tensor_mul(
                    qn[:, ic, hh * HWH : (hh + 1) * HWH],
                    x1_t[:, ic, hh * HWH : (hh + 1) * HWH],
                    rep[:, hh, :],
                )
        # group g's rows live at partitions 32g..32g+1 (32-aligned so the
        # epilogue engine reads are legal); dot and ss2 in separate tiles
        pdot = psum_pool.tile([3 * 32 + SPG, 2, HWH], FP32)  # 2 banks
        pss2 = psum2_pool.tile([3 * 32 + SPG, 2, HWH], FP32)  # 2 banks
        return pdot, pss2, qn

    def epilogue(b, pdot, pss2, g, hh_list):
        # group g: dist = 1 - dot * rsqrt(ss2) for s in {g, g+4}
        nh = len(hh_list)
        h0 = hh_list[0]
        rows = slice(32 * g, 32 * g + SPG)
        rr2 = rr2_pool.tile([SPG, nh, HWH], FP32)
        nc.scalar.activation(rr2[:], pss2[rows, h0 : h0 + nh, :], func=RSQRT)
        t = t_pool.tile([SPG, nh, HWH], FP32)
        nc.vector.tensor_mul(t[:], pdot[rows, h0 : h0 + nh, :], rr2[:])
        dist = dist_pool.tile([SPG, nh, HWH], FP32)
        # 1 - t on ScalarE (Copy computes in*scale + bias); keeps DVE free
        nc.scalar.activation(
            dist[:], t[:], func=mybir.ActivationFunctionType.Copy, scale=-1.0, bias=1.0
        )
        nc.sync.dma_start(
            outr[b, g][:, h0 * HWH : (h0 + nh) * HWH], dist[:]
        )

    x1_t = load_x1(0)
    cur = qn_chain(0, x1_t)
    nxt_x1 = None
    nxt = None
    for b in range(BL):
        pdot, pss2, qn = cur
        for s in range(S):
            g = s % NG
            j = s // NG
            x2_t = x2_pool.tile([P, NCH, HW], BF16)
            last = b == BL - 1 and s == S - 1
            if last:
                # split the final load so the tail drains per hw half
                for hh in range(2):
                    nc.gpsimd.dma_start(
                        x2_t[:, :, hh * HWH : (hh + 1) * HWH],
                        x2r[b, s][:, :, hh * HWH : (hh + 1) * HWH],
                    )
            else:
                nc.gpsimd.dma_start(x2_t[:], x2r[b, s])
            if s == 0 and b + 1 < BL:
                nxt_x1 = load_x1(b + 1)

            if last:
                prods = []
                for hh in range(2):
                    hsl = slice(hh * HWH, (hh + 1) * HWH)
                    prod = prod_pool.tile([P, NCH, HWH], BF16)
                    nc.vector.tensor_mul(prod[:], qn[:, :, hsl], x2_t[:, :, hsl])
                    sq2 = sq2_pool.tile([P, NCH, HWH], BF16)
                    nc.scalar.activation(sq2[:], x2_t[:, :, hsl], func=SQUARE)
                    prods.append((prod, sq2))
            else:
                prod = prod_pool.tile([P, NCH, HW], BF16)
                nc.vector.tensor_mul(prod[:], qn[:], x2_t[:])
                sq2 = sq2_pool.tile([P, NCH, HW], BF16)
                if s == 3:
                    # balance: DVE picks up one square pass per batch
                    nc.vector.tensor_mul(sq2[:], x2_t[:], x2_t[:])
                else:
                    nc.scalar.activation(sq2[:], x2_t[:], func=SQUARE)

            # group g accumulates row j at partitions 32g..32g+1
            rows = slice(32 * g, 32 * g + SPG)

            def mm(kind, hh):
                if last:
                    src = prods[hh][kind]
                    csl = slice(0, HWH)
                else:
                    src = prod if kind == 0 else sq2
                    csl = slice(hh * HWH, (hh + 1) * HWH)
                acc = pdot if kind == 0 else pss2
                for ic in range(NCH):
                    nc.tensor.matmul(
                        acc[rows, hh, :],
                        oh2[:, j, :],
                        src[:, ic, csl],
                        start=(j == 0 and ic == 0),
                        stop=(j == SPG - 1 and ic == NCH - 1),
                        tile_position=(0, 32 * g),
                    )

            if last:
                for hh in range(2):
                    mm(0, hh)
                    mm(1, hh)
                    epilogue(b, pdot, pss2, g, [hh])
            else:
                # dot for both halves first (prod is ready before sq2)
                mm(0, 0)
                mm(0, 1)
                mm(1, 0)
                mm(1, 1)
                if j == SPG - 1:
                    epilogue(b, pdot, pss2, g, [0, 1])

            if s == 1 and b + 1 < BL:
                nxt = qn_chain(b + 1, nxt_x1)

        cur = nxt


def _build():
    # Bacc (not plain Bass): its compile pipeline legalizes TRN2's
    # one-sync-wait-per-instruction limit (generate_event_semaphores).
    nc = bacc.Bacc("TRN2")
    x1 = nc.dram_tensor("x1", [BL, C, HW], FP32, kind="ExternalInput")
    x2 = nc.dram_tensor("x2", [BL, S, C, HW], FP32, kind="ExternalInput")
    out = nc.dram_tensor("out", [BL, S, HW], FP32, kind="ExternalOutput")
    with tile.TileContext(nc) as tc:
        with ExitStack() as ctx:
            _emit(ctx, tc, x1[:], x2[:], out[:])
    nc.finalize()
    return nc


_NC = None

# test-harness knobs (the grading harness never touches these)
TRACE = False
TRACE_DIR = None
LAST_RESULTS = None


def _get_nc():
    global _NC
    if _NC is None:
        _NC = _build()
    return _NC


def kernel(x1: np.ndarray, x2: np.ndarray) -> np.ndarray:
    global LAST_RESULTS
    x1 = np.ascontiguousarray(x1, dtype=np.float32).reshape(B, C, HW)
    x2 = np.ascontiguousarray(x2, dtype=np.float32).reshape(B, S, C, HW)
    nc = _get_nc()
    in_maps = [
        {"x1": x1[c * BL : (c + 1) * BL], "x2": x2[c * BL : (c + 1) * BL]}
        for c in range(N_CORES)
    ]
    res = run_bass_kernel_spmd(
        nc, in_maps, list(range(N_CORES)), trace=TRACE, tmpdir=TRACE_DIR
    )
    LAST_RESULTS = res
    outs = [res.results[c]["out"].reshape(BL, S * HW) for c in range(N_CORES)]
    return np.concatenate(outs, axis=0)


# revision 16
# speedup vs baseline: 1.0399x; 1.0399x over previous
"""Trainium2 Bass kernel for per-pixel cosine-distance block.

x1: [B, C, h, w]  f32
x2: [B, S, C, h, w] f32
out: [B, S*h*w] f32  where out[b, s*h*w + p] = 1 - cos(x1[b,:,p], x2[b,s,:,p])
(cosine over the channel dim C, per pixel)

Sharding: data-parallel over B across 8 NeuronCores (4 batches per core).

Per-core pipeline (C=512 on partitions as 4 chunks of 128, hw=1024 on free;
c = p*4+k so each partition's chunks are 16 KiB-contiguous in DRAM):

  All input DMAs are SWDGE (gpsimd) f32->bf16 cast loads on one queue at
  HBM line rate (~425 GB/s/core read side); the gpsimd engine issues
  nothing else, so the load queue is never blocked behind compute.
  Output stores ride the otherwise-idle sync HWDGE queue.

  x1 is normalized up front per batch (qn = x1 * rsqrt(sum x1^2), with the
  rsqrt row broadcast across partitions by a K=1 matmul), so the per-tile
  work is: prod = qn*x2 (DVE bf16 2x mode), sq2 = x2^2 (ScalarE; DVE for
  one s per batch to balance the two engines), and one-hot matmul
  partition-reductions into PSUM.

  TensorE: s-tile s uses PE column group g = s%4 (tile_position=(0,32g)).
  Group g accumulates rows [dot[g], dot[g+4], ss2[g], ss2[g+4]] at PSUM
  partitions 32g..32g+3 via an M=4 one-hot lhsT. Column groups execute
  concurrently when several tiles' matmuls are pending, so the PE keeps
  up with the DMA cadence even at its cold (1.2 GHz) clock, and each
  group's accumulation closes at s=g+4 - epilogues and output stores
  spread through the batch instead of piling up at the end. The final
  drain after the last byte lands is just group 3's short chain
  (rsqrt -> mul -> 1-x -> store), with the last tile loaded in two hw
  halves to shorten it further.
"""

from contextlib import ExitStack

import numpy as np

import concourse.bass as bass
import concourse.tile as tile
from concourse import bacc, mybir
from concourse.bass_utils import run_bass_kernel_spmd

B, S, C, H, W = 32, 8, 512, 32, 32
HW = H * W  # 1024
N_CORES = 8
BL = B // N_CORES  # 4 batches per core
P = 128
NCH = C // P  # 4 chunks of the channel dim
HWH = HW // 2  # 512 (one PSUM bank of f32)
NG = 4  # PE column groups; s-tile s -> group s % NG
SPG = S // NG  # s values per group (2)

FP32 = mybir.dt.float32
BF16 = mybir.dt.bfloat16

RSQRT = mybir.ActivationFunctionType.Abs_reciprocal_sqrt
SQUARE = mybir.ActivationFunctionType.Square


def _emit(ctx: ExitStack, tc: tile.TileContext, x1, x2, out):
    nc = tc.nc

    # c = p*NCH + k -> partition p, chunk k: 16 KiB contiguous per partition
    x1r = x1.rearrange("b (p k) f -> b p k f", p=P)  # [BL, 128, NCH, HW]
    x2r = x2.rearrange("b s (p k) f -> b s p k f", p=P)  # [BL, S, 128, NCH, HW]
    # s = j*NG + g -> out rows of group g hold s in {g, g+4}
    outr = out.rearrange("b (j g) f -> b g j f", g=NG)  # [BL, NG, SPG, HW]

    singles = ctx.enter_context(tc.tile_pool(name="singles", bufs=1))
    x1_pool = ctx.enter_context(tc.tile_pool(name="x1", bufs=2))
    qn_pool = ctx.enter_context(tc.tile_pool(name="qn", bufs=2))
    sq1_pool = ctx.enter_context(tc.tile_pool(name="sq1", bufs=1))
    x2_pool = ctx.enter_context(tc.tile_pool(name="x2", bufs=10))
    prod_pool = ctx.enter_context(tc.tile_pool(name="prod", bufs=3))
    sq2_pool = ctx.enter_context(tc.tile_pool(name="sq2", bufs=3))
    rr1_pool = ctx.enter_context(tc.tile_pool(name="rr1", bufs=1))
    rr2_pool = ctx.enter_context(tc.tile_pool(name="rr2", bufs=2))
    t_pool = ctx.enter_context(tc.tile_pool(name="tp", bufs=2))
    dist_pool = ctx.enter_context(tc.tile_pool(name="dist", bufs=2))
    # PSUM: 8 banks exactly = dot acc (2) + ss2 acc (2) + ss1 (2) + rep (2).
    # bufs=1 is safe: each group's accumulator is read (epilogue) at s=g+4
    # of batch b, ~4 tiles before batch b+1 rewrites it at its s=g.
    psum_pool = ctx.enter_context(tc.tile_pool(name="pacc", bufs=1, space="PSUM"))
    psum2_pool = ctx.enter_context(tc.tile_pool(name="pac2", bufs=1, space="PSUM"))
    ss1_pool = ctx.enter_context(tc.tile_pool(name="ss1p", bufs=1, space="PSUM"))
    rep_pool = ctx.enter_context(tc.tile_pool(name="rep", bufs=1, space="PSUM"))

    # oh2[:, j, :] is a [P, 2] matrix, all-ones in column j: as lhsT it
    # deposits the partition-reduction of rhs into row j of the 2-row
    # group region (adding zero to the other row).
    oh2 = singles.tile([P, SPG, SPG], BF16)
    nc.vector.memset(oh2, 0.0)
    for r in range(SPG):
        nc.vector.memset(oh2[:, r, r : r + 1], 1.0)
    ones1 = singles.tile([P, 1], BF16)
    nc.vector.memset(ones1, 1.0)
    # [1, P] ones: K=1 matmul with it as lhsT replicates an SBUF row across
    # all 128 PSUM partitions (fp32 so the values pass through unchanged).
    ones128 = singles.tile([1, P], FP32)
    nc.vector.memset(ones128, 1.0)

    def load_x1(b):
        x1_t = x1_pool.tile([P, NCH, HW], BF16)
        nc.gpsimd.dma_start(x1_t[:], x1r[b])
        return x1_t

    def qn_chain(b, x1_t):
        # qn = x1 * rsqrt(sum_c x1^2); also allocates batch b's group
        # accumulator (partitions 32g..32g+3: dot[g], dot[g+4], ss2[g],
        # ss2[g+4], each [2 banks] = both hw halves).
        sq1 = sq1_pool.tile([P, NCH, HW], BF16)
        nc.vector.tensor_mul(sq1[:], x1_t[:], x1_t[:])
        ss1 = ss1_pool.tile([1, 2, HWH], FP32)
        for hh in range(2):
            for ic in range(NCH):
                nc.tensor.matmul(
                    ss1[:, hh, :],
                    ones1,
                    sq1[:, ic, hh * HWH : (hh + 1) * HWH],
                    start=(ic == 0),
                    stop=(ic == NCH - 1),
                )
        rr1 = rr1_pool.tile([1, 2, HWH], FP32)
        nc.scalar.activation(rr1[:], ss1[:], func=RSQRT)
        rep = rep_pool.tile([P, 2, HWH], FP32)  # 2 banks
        for hh in range(2):
            nc.tensor.matmul(
                rep[:, hh, :], ones128, rr1[:, hh, :], start=True, stop=True
            )
        qn = qn_pool.tile([P, NCH, HW], BF16)
        for ic in range(NCH):
            for hh in range(2):
                nc.vector.tensor_mul(
                    qn[:, ic, hh * HWH : (hh + 1) * HWH],
                    x1_t[:, ic, hh * HWH : (hh + 1) * HWH],
                    rep[:, hh, :],
                )
        # group g's rows live at partitions 32g..32g+1 (32-aligned so the
        # epilogue engine reads are legal); dot and ss2 in separate tiles
        pdot = psum_pool.tile([3 * 32 + SPG, 2, HWH], FP32)  # 2 banks
        pss2 = psum2_pool.tile([3 * 32 + SPG, 2, HWH], FP32)  # 2 banks
        return pdot, pss2, qn

    NPR = 3 * 32 + SPG  # 98: all four group regions in one partition span

    def epilogue(b, pdot, pss2, hh_list):
        # dist = 1 - dot * rsqrt(ss2), all four groups in ONE op each:
        # engine ops are free-dim bound, so the 92 in-between partitions
        # compute garbage for free (their lanes run in parallel anyway).
        nh = len(hh_list)
        h0 = hh_list[0]
        rr2 = rr2_pool.tile([NPR, nh, HWH], FP32)
        nc.scalar.activation(rr2[:], pss2[0:NPR, h0 : h0 + nh, :], func=RSQRT)
        t = t_pool.tile([NPR, nh, HWH], FP32)
        nc.vector.tensor_mul(t[:], pdot[0:NPR, h0 : h0 + nh, :], rr2[:])
        dist = dist_pool.tile([NPR, nh, HWH], FP32)
        # 1 - t on ScalarE (Copy computes in*scale + bias); keeps DVE free
        nc.scalar.activation(
            dist[:], t[:], func=mybir.ActivationFunctionType.Copy, scale=-1.0, bias=1.0
        )
        for g in range(NG):
            nc.sync.dma_start(
                outr[b, g][:, h0 * HWH : (h0 + nh) * HWH],
                dist[32 * g : 32 * g + SPG],
            )

    x1_t = load_x1(0)
    cur = qn_chain(0, x1_t)
    nxt_x1 = None
    nxt = None
    for b in range(BL):
        pdot, pss2, qn = cur
        for s in range(S):
            g = s % NG
            j = s // NG
            x2_t = x2_pool.tile([P, NCH, HW], BF16)
            last = b == BL - 1 and s == S - 1
            if last:
                # split the final load so the tail drains per hw half
                for hh in range(2):
                    nc.gpsimd.dma_start(
                        x2_t[:, :, hh * HWH : (hh + 1) * HWH],
                        x2r[b, s][:, :, hh * HWH : (hh + 1) * HWH],
                    )
            else:
                nc.gpsimd.dma_start(x2_t[:], x2r[b, s])
            if s == 0 and b + 1 < BL:
                nxt_x1 = load_x1(b + 1)

            if last:
                prods = []
                for hh in range(2):
                    hsl = slice(hh * HWH, (hh + 1) * HWH)
                    prod = prod_pool.tile([P, NCH, HWH], BF16)
                    nc.vector.tensor_mul(prod[:], qn[:, :, hsl], x2_t[:, :, hsl])
                    sq2 = sq2_pool.tile([P, NCH, HWH], BF16)
                    nc.scalar.activation(sq2[:], x2_t[:, :, hsl], func=SQUARE)
                    prods.append((prod, sq2))
            else:
                prod = prod_pool.tile([P, NCH, HW], BF16)
                nc.vector.tensor_mul(prod[:], qn[:], x2_t[:])
                sq2 = sq2_pool.tile([P, NCH, HW], BF16)
                if s == 3:
                    # balance: DVE picks up one square pass per batch
                    nc.vector.tensor_mul(sq2[:], x2_t[:], x2_t[:])
                else:
                    nc.scalar.activation(sq2[:], x2_t[:], func=SQUARE)

            # group g accumulates row j at partitions 32g..32g+1
            rows = slice(32 * g, 32 * g + SPG)

            def mm(kind, hh):
                if last:
                    src = prods[hh][kind]
                    csl = slice(0, HWH)
                else:
                    src = prod if kind == 0 else sq2
                    csl = slice(hh * HWH, (hh + 1) * HWH)
                acc = pdot if kind == 0 else pss2
                for ic in range(NCH):
                    nc.tensor.matmul(
                        acc[rows, hh, :],
                        oh2[:, j, :],
                        src[:, ic, csl],
                        start=(j == 0 and ic == 0),
                        stop=(j == SPG - 1 and ic == NCH - 1),
                        tile_position=(0, 32 * g),
                    )

            if last:
                for hh in range(2):
                    mm(0, hh)
                    mm(1, hh)
                    epilogue(b, pdot, pss2, [hh])
            elif s == S - 1:
                # dot for both halves first (prod is ready before sq2)
                mm(0, 0)
                mm(0, 1)
                mm(1, 0)
                mm(1, 1)
                epilogue(b, pdot, pss2, [0, 1])
            else:
                mm(0, 0)
                mm(0, 1)
                mm(1, 0)
                mm(1, 1)

            if s == 1 and b + 1 < BL:
                nxt = qn_chain(b + 1, nxt_x1)

        cur = nxt


def _build():
    # Bacc (not plain Bass): its compile pipeline legalizes TRN2's
    # one-sync-wait-per-instruction limit (generate_event_semaphores).
    nc = bacc.Bacc("TRN2")
    x1 = nc.dram_tensor("x1", [BL, C, HW], FP32, kind="ExternalInput")
    x2 = nc.dram_tensor("x2", [BL, S, C, HW], FP32, kind="ExternalInput")
    out = nc.dram_tensor("out", [BL, S, HW], FP32, kind="ExternalOutput")
    with tile.TileContext(nc) as tc:
        with ExitStack() as ctx:
            _emit(ctx, tc, x1[:], x2[:], out[:])
    nc.finalize()
    return nc


_NC = None

# test-harness knobs (the grading harness never touches these)
TRACE = False
TRACE_DIR = None
LAST_RESULTS = None


def _get_nc():
    global _NC
    if _NC is None:
        _NC = _build()
    return _NC


def kernel(x1: np.ndarray, x2: np.ndarray) -> np.ndarray:
    global LAST_RESULTS
    x1 = np.ascontiguousarray(x1, dtype=np.float32).reshape(B, C, HW)
    x2 = np.ascontiguousarray(x2, dtype=np.float32).reshape(B, S, C, HW)
    nc = _get_nc()
    in_maps = [
        {"x1": x1[c * BL : (c + 1) * BL], "x2": x2[c * BL : (c + 1) * BL]}
        for c in range(N_CORES)
    ]
    res = run_bass_kernel_spmd(
        nc, in_maps, list(range(N_CORES)), trace=TRACE, tmpdir=TRACE_DIR
    )
    LAST_RESULTS = res
    outs = [res.results[c]["out"].reshape(BL, S * HW) for c in range(N_CORES)]
    return np.concatenate(outs, axis=0)
